# revision 56
# baseline (speedup 1.0000x reference)
"""MinibatchDiscrimination Trainium2 kernel v3 (8-core SPMD, full I/O).

Math (reference):
  act = einsum('bd,kdm->bkm', x, W)        # (512, 64, 16)
  l1[i,j,k] = sum_m |act[i,k,m] - act[j,k,m]|
  feats[i,k] = sum_j exp(-l1[i,j,k]) + b[k]
  out = concat([x, feats], axis=1)         # (512, 320)

v3 strategy (vs v2: bf16 end-to-end, DVE 4x mode, warm PE):
  - |a-b| = a + b - 2*min(a,b): the pairwise elementwise op becomes a SINGLE
    DVE tensor_scalar min (all-bf16 SBUF step-1 -> 4x_2P mode, ~127ns/tile
    vs 331ns for v2's 1x fp8 path). Each output kernel k draws its 16 m
    values from exactly one km-tile (k = 8t + p//16), so the identity can be
    applied per-tile: tiles t<7 carry min values with +1 matmul weights and
    the S-corrections; tile t=7 goes through ACT's fused Abs(x - row) with
    -0.5 weights and no S-correction (S^B[k>=56] = 0 by construction).
    exp arg: 2*pl - S^B_i where pl = sum_B min - 0.5*sum_A |d| - 0.5*S^B_j
    (the -0.5*S^B_j folded in by one extra idB matmul per pair).
  - W=256 windows, both rows of a pair 4B-aligned: e0=[2q,2q+256) (self incl,
    d=1..255), e1=[2q+2,2q+258) (d=1..256). Even-row d=256 pairs {2i,2i+256}
    via a small all-min block (full-S corrections); odd-row d=256 dups
    masked on cores 4-7 (-30 on min tiles / +30 on the abs tile, col 255).
  - PE: per pair 8 gdr matmuls at FD=512 ([64k, 2e, 256w] = one PSUM bank)
    + 1 fused wj matmul (hS2 holds both shifted window halves), tile-major
    supers of 7 pairs to amortize weight loads and keep the PE busy so the
    HAM clock gate stays at 8/8 (2.4 GHz) instead of v2's permanent 4/8
    throttle (throttle_active: 134us -> ~20us).
  - ACT: exp(2*pl - S^B_i) with accum_out row sums; 2 abs tiles/pair; the
    two mask ops per pair (Copy with +-30 bias on strided col views).
  - Column sums via identity-matmul accumulation into a persistent PSUM
    bank; host scatters rotated partials and adds the exact +1 self term.
"""

import sys

sys.path.insert(0, "/opt/trn_rl_repo")

import numpy as np
import ml_dtypes

import concourse.bass as bass
import concourse.bacc as bacc
import concourse.tile as tile
from concourse import mybir
from concourse import bass_utils

B, D, K, M = 512, 256, 64, 16
KM = K * M          # 1024
NT = KM // 128      # 8 km-tiles
NA = 1              # abs-path km-tiles (handled by ACT), t >= NT-NA
NCORES = 8
RPC = B // NCORES   # 64 rows per core
NPAIR = RPC // 2    # 32 pairs per core
W = 256             # window width per row
EXT = 320           # extended actT columns (64 + 256)
ACC = 320           # column accumulator width
ND = RPC            # d256 block columns (all local rows)
MASKV = 30.0        # |pre-l1 bump| for the masked e1 d=256 column

FP32 = mybir.dt.float32
BF16 = mybir.dt.bfloat16

# supers: groups of pairs sharing a PSUM-bank generation; 4 pairs per super
# with a 7-buffer pool keeps three spare banks so the next super's first
# pairs never wait on the previous super's exps
SUPERS = [list(range(4 * s, min(4 * s + 4, NPAIR))) for s in range(8)]


def build_bass():
    nc = bacc.Bacc(None, target_bir_lowering=False, debug=False)

    xTr = nc.declare_dram_parameter("xTr", [D, EXT], BF16, isOutput=False)
    w2 = nc.declare_dram_parameter("w2", [D, KM], BF16, isOutput=False)
    xi = nc.declare_dram_parameter("xi", [RPC, D], FP32, isOutput=False)
    brep = nc.declare_dram_parameter("brep", [NPAIR, 2 * K], FP32, isOutput=False)
    gdr = nc.declare_dram_parameter("gdr", [128, NT, K], BF16, isOutput=False)
    gdr2 = nc.declare_dram_parameter("gdr2", [128, NT, K], BF16, isOutput=False)
    identB = nc.declare_dram_parameter("identB", [64, 64], BF16, isOutput=False)
    identF = nc.declare_dram_parameter("identF", [128, 128], FP32, isOutput=False)
    biasD = nc.declare_dram_parameter("biasD", [64, 1], FP32, isOutput=False)
    out = nc.declare_dram_parameter("out", [RPC, D + K], FP32, isOutput=True)
    outc = nc.declare_dram_parameter("outc", [K, ACC], FP32, isOutput=True)

    with tile.TileContext(nc) as tc:
        with (
            tc.tile_pool(name="consts", bufs=1) as consts,
            tc.tile_pool(name="work", bufs=11) as work,
            tc.tile_pool(name="small", bufs=10) as small,
            tc.tile_pool(name="psum_l", bufs=7, space="PSUM") as psum_l,
            tc.tile_pool(name="psum_c", bufs=1, space="PSUM") as psum_c,
        ):
            # ---- load inputs ----
            xTr_b = consts.tile([128, 2, EXT], BF16, tag="xTr_b")
            nc.sync.dma_start(out=xTr_b, in_=xTr[:].rearrange("(h p) b -> p h b", p=128))
            w2_b = consts.tile([128, 2, KM], BF16, tag="w2_b")
            nc.sync.dma_start(out=w2_b, in_=w2[:].rearrange("(h p) n -> p h n", p=128))
            xi_f = consts.tile([RPC, D], FP32, tag="xi_f")
            nc.sync.dma_start(out=xi_f, in_=xi[:])
            brep_f = consts.tile([NPAIR, 2 * K], FP32, tag="brep_f")
            nc.sync.dma_start(out=brep_f, in_=brep[:])
            gdr_b = consts.tile([128, NT, K], BF16, tag="gdr_b")
            nc.sync.dma_start(out=gdr_b, in_=gdr[:])
            gdr2_b = consts.tile([128, NT, K], BF16, tag="gdr2_b")
            nc.sync.dma_start(out=gdr2_b, in_=gdr2[:])
            idB = consts.tile([64, 64], BF16, tag="idB")
            nc.sync.dma_start(out=idB, in_=identB[:])
            idF = consts.tile([128, 128], FP32, tag="idF")
            nc.sync.dma_start(out=idF, in_=identF[:])
            biasD_b = consts.tile([64, 1], FP32, tag="biasD_b")
            nc.sync.dma_start(out=biasD_b, in_=biasD[:])

            # ---- prologue: actTx [128, NT, EXT] bf16; row values f32 ----
            actTx = consts.tile([128, NT, EXT], BF16, tag="actTx")
            posIT = consts.tile([128, NT, RPC], FP32, tag="posIT")
            negIT7 = consts.tile([128, NA, RPC], FP32, tag="negIT7")
            for t in range(NT):
                pb = psum_l.tile([128, 512], FP32, tag="pb")
                pa = pb[:, 0:EXT]
                for dh in range(2):
                    nc.tensor.matmul(
                        pa,
                        w2_b[:, dh, t * 128:(t + 1) * 128],
                        xTr_b[:, dh, :],
                        start=(dh == 0),
                        stop=(dh == 1),
                    )
                # all copies on ACT: keeps the DVE free to start min ops
                nc.scalar.copy(actTx[:, t, :], pa)
                if t >= NT - NA:
                    nc.vector.tensor_scalar(
                        out=negIT7[:, t - (NT - NA), :], in0=actTx[:, t, 0:RPC],
                        scalar1=-1.0, scalar2=None, op0=mybir.AluOpType.mult,
                    )
                else:
                    nc.vector.tensor_scalar(
                        out=posIT[:, t, :], in0=actTx[:, t, 0:RPC],
                        scalar1=1.0, scalar2=None, op0=mybir.AluOpType.mult,
                    )

            # ---- S sums: SnegB = -S^B (f32, exp bias), halves in bf16 ----
            # S^B[k, c] = sum_m act[c, k, m] over min-path tiles (k < 56 only)
            SnegB = consts.tile([K, EXT], FP32, tag="SnegB")
            hS2 = consts.tile([K, 2, EXT], BF16, tag="hS2")
            halfSnegF = consts.tile([K, EXT], BF16, tag="halfSnegF")
            pbS = psum_l.tile([128, 512], FP32, tag="pb")
            pS = pbS[0:K, 0:EXT]
            for t in range(NT - NA):
                nc.tensor.matmul(pS, gdr2_b[:, t, :], actTx[:, t, :],
                                 start=(t == 0), stop=False)
            nc.scalar.activation(out=SnegB, in_=pS,
                                 func=mybir.ActivationFunctionType.Copy,
                                 scale=-1.0)
            # hS2[:, e, c] = -0.5 * S^B[c + 2e]: both window halves of the
            # wj term, readable as one [64, 2, 256] moving operand per pair
            nc.scalar.activation(out=hS2[:, 0, :], in_=pS,
                                 func=mybir.ActivationFunctionType.Copy,
                                 scale=-0.5)
            nc.scalar.activation(out=hS2[:, 1, 0:EXT - 2], in_=pS[:, 2:EXT],
                                 func=mybir.ActivationFunctionType.Copy,
                                 scale=-0.5)
            for t in range(NT - NA, NT):
                nc.tensor.matmul(pS, gdr2_b[:, t, :], actTx[:, t, :],
                                 start=False, stop=(t == NT - 1))
            nc.scalar.activation(out=halfSnegF, in_=pS,
                                 func=mybir.ActivationFunctionType.Copy,
                                 scale=-0.5)

            # zero tile for the col-accumulator group opener
            zt = consts.tile([64, ACC], BF16, tag="zt")
            nc.vector.memset(zt, 0.0)

            # ---- column accumulator PSUM bank + d256 partial ----
            accP = psum_c.tile([64, ACC + 2 * ND], FP32, tag="accP")
            acc2 = accP[:, 0:ACC]
            plB = accP[:, ACC + ND:ACC + 2 * ND]
            nc.tensor.matmul(acc2, idB, zt, start=True, stop=False,
                             skip_group_check=True)

            # d256 block tiles (ops emitted after super 0's mins, used at s=1)
            dif256 = consts.tile([128, NT, ND], BF16, tag="dif256")
            dSb = consts.tile([K, ND], BF16, tag="dSb")

            featsP0 = consts.tile([K, NPAIR], FP32, tag="featsP0")
            featsP1 = consts.tile([K, NPAIR], FP32, tag="featsP1")
            featsP = [featsP0, featsP1]
            accF = consts.tile([K, ACC], FP32, tag="accF")
            scrB = consts.tile([K, ND], BF16, tag="scrB")

            # ---- main loop over supers ----
            adif_of = {}
            scr_of = {}
            prev_super = None
            for s, sup in enumerate(SUPERS):
                # DVE: min(actTx, row) on tiles t<7; ACT: |actTx-row| on t=7
                for q in sup:
                    lo = 2 * q
                    adif = work.tile([128, NT, 2, W], BF16, tag="adif")
                    adif_of[q] = adif
                    for e in range(2):
                        il = lo + e
                        for t in range(NT - NA):
                            nc.vector.tensor_scalar(
                                out=adif[:, t, e, :],
                                in0=actTx[:, t, lo + 2 * e:lo + 2 * e + W],
                                scalar1=posIT[:, t, il:il + 1],
                                scalar2=None,
                                op0=mybir.AluOpType.min,
                            )
                        for t in range(NT - NA, NT):
                            nc.scalar.activation(
                                out=adif[:, t, e, :],
                                in_=actTx[:, t, lo + 2 * e:lo + 2 * e + W],
                                func=mybir.ActivationFunctionType.Abs,
                                bias=negIT7[:, t - (NT - NA), il:il + 1],
                                scale=1.0,
                            )
                    # mask e0 col 0 (self; host adds exact +1) and e1 col 255
                    # (d=256, handled by the d256 block) on all cores:
                    # min tiles get -30 (exp arg -480), abs tile +30
                    mv = adif[:, 0:NT - NA, :, :].rearrange(
                        "p t e w -> p t (e w)")
                    nc.scalar.activation(
                        out=mv[:, :, 0:2 * W:2 * W - 1],
                        in_=mv[:, :, 0:2 * W:2 * W - 1],
                        func=mybir.ActivationFunctionType.Copy,
                        bias=-MASKV, scale=1.0,
                    )
                    ma = adif[:, NT - NA:NT, :, :].rearrange(
                        "p t e w -> p t (e w)")
                    nc.scalar.activation(
                        out=ma[:, :, 0:2 * W:2 * W - 1],
                        in_=ma[:, :, 0:2 * W:2 * W - 1],
                        func=mybir.ActivationFunctionType.Copy,
                        bias=MASKV, scale=1.0,
                    )

                if s == 0:
                    # d256 DVE prep: all-min with full-S corrections
                    # (plB = sum_m min - 0.5(Se+So)); emitted after super 0's
                    # mins so they don't delay the pipeline ramp
                    for t in range(NT):
                        nc.vector.tensor_tensor(
                            out=dif256[:, t, :],
                            in0=actTx[:, t, 0:RPC],
                            in1=actTx[:, t, W:W + RPC],
                            op=mybir.AluOpType.min,
                        )
                    nc.vector.tensor_tensor(
                        out=dSb, in0=halfSnegF[:, 0:RPC],
                        in1=halfSnegF[:, W:W + RPC], op=mybir.AluOpType.add,
                    )

                # PE: diagonal schedule — pair qi runs tile t at step qi+t,
                # closes (wj matmul, stop) at step qi+NT, exp fires right
                # after, so group completions and exps stagger instead of
                # bursting at the super boundary.
                pl_of = {}
                for q in sup:
                    pb = psum_l.tile([128, 512], FP32, tag="pb")
                    pl_of[q] = pb[0:K, :].rearrange("p (e w) -> p e w", e=2)
                for step in range(len(sup) + NT + 1):
                    for qi, q in enumerate(sup):
                        t = step - qi
                        lo = 2 * q
                        if 0 <= t < NT:
                            nc.tensor.matmul(
                                pl_of[q],
                                gdr_b[:, t, :],
                                adif_of[q][:, t, :, :],
                                start=(t == 0),
                                stop=False,
                            )
                        elif t == NT:
                            # wj matmul closes the group; exp consumes it
                            nc.tensor.matmul(
                                pl_of[q], idB, hS2[:, :, lo:lo + W],
                                start=False, stop=True,
                            )
                            scr = small.tile([K, 2, W], BF16, tag="scr")
                            scr_of[q] = scr
                            # last supers: row sums on the tail-idle DVE
                            # instead of ACT's accumulator+READ path
                            tail = s >= len(SUPERS) - 2
                            for e in range(2):
                                nc.scalar.activation(
                                    out=scr[:, e, :], in_=pl_of[q][:, e, :],
                                    func=mybir.ActivationFunctionType.Exp,
                                    bias=SnegB[:, lo + e:lo + e + 1],
                                    scale=2.0,
                                    accum_out=(None if tail else
                                               featsP[e][:, q:q + 1]),
                                )
                                if tail:
                                    nc.vector.tensor_scalar(
                                        out=scr[:, e, :], in0=scr[:, e, :],
                                        scalar1=1.0, scalar2=0.0,
                                        op0=mybir.AluOpType.mult,
                                        op1=mybir.AluOpType.add,
                                        accum_out=featsP[e][:, q:q + 1],
                                    )
                    if step == 0 and s == 1:
                        # d256 block PE + ACT (once, early)
                        for tt in range(NT):
                            nc.tensor.matmul(
                                plB, gdr2_b[:, tt, :], dif256[:, tt, :],
                                start=(tt == 0), stop=False,
                                skip_group_check=True,
                            )
                        nc.tensor.matmul(
                            plB, idB, dSb,
                            start=False, stop=True, skip_group_check=True,
                        )
                        nc.scalar.activation(
                            out=scrB, in_=plB,
                            func=mybir.ActivationFunctionType.Exp,
                            bias=biasD_b[:, 0:1], scale=2.0,
                        )
                    if step == 1 and prev_super is not None:
                        for qp in prev_super:
                            lop = 2 * qp
                            for e in range(2):
                                nc.tensor.matmul(
                                    acc2[:, lop + 2 * e:lop + 2 * e + W],
                                    idB, scr_of[qp][:, e, :],
                                    start=False, stop=False,
                                    skip_group_check=True,
                                )
                prev_super = sup

            # colaccs of the final super
            last = SUPERS[-1]
            for qp in last:
                lop = 2 * qp
                for e in range(2):
                    nc.tensor.matmul(
                        acc2[:, lop + 2 * e:lop + 2 * e + W],
                        idB, scr_of[qp][:, e, :],
                        start=False, stop=False,
                        skip_group_check=True,
                    )
            # d256 contributions: rows 64c+i (cols i) and 64c+256+i (cols 256+i)
            nc.tensor.matmul(
                acc2[:, 0:RPC], idB, scrB,
                start=False, stop=False, skip_group_check=True,
            )
            nc.tensor.matmul(
                acc2[:, W:W + RPC], idB, scrB,
                start=False, stop=True, skip_group_check=True,
            )

            # ---- outputs ----
            nc.scalar.copy(accF, acc2)
            nc.sync.dma_start(out=outc[:], in_=accF)

            outf = consts.tile([NPAIR, 2 * K], FP32, tag="outf")
            for e in range(2):
                pb = psum_l.tile([128, 512], FP32, tag="pb")
                ptr = pb[0:NPAIR, 0:K]
                nc.tensor.transpose(ptr, featsP[e], idF[0:K, 0:K])
                nc.vector.tensor_tensor(
                    out=outf[:, e * K:(e + 1) * K], in0=ptr,
                    in1=brep_f[:, e * K:(e + 1) * K], op=mybir.AluOpType.add,
                )
            nc.sync.dma_start(
                out=out[:, D:D + K].rearrange("(c e) k -> c e k", e=2),
                in_=outf.rearrange("c (e k) -> c e k", e=2),
            )
            nc.sync.dma_start(out=out[:, 0:D], in_=xi_f)

    nc.compile()
    return nc


_NC_CACHE = None


def _get_nc():
    global _NC_CACHE
    if _NC_CACHE is None:
        _NC_CACHE = build_bass()
    return _NC_CACHE


def make_in_maps(x, W_, b):
    x = np.asarray(x, dtype=np.float32)
    W_ = np.asarray(W_, dtype=np.float32)
    b = np.asarray(b, dtype=np.float32)
    xT = np.ascontiguousarray(x.T)                       # (256, 512) fp32
    w2 = np.ascontiguousarray(
        W_.transpose(1, 0, 2).reshape(D, KM)).astype(ml_dtypes.bfloat16)
    brep = np.ascontiguousarray(
        np.broadcast_to(np.tile(b, 2)[None, :], (NPAIR, 2 * K)))

    # indicator[p, t, k] = 1 iff k == 8*t + p//16
    p = np.arange(128)[:, None, None]
    t = np.arange(NT)[None, :, None]
    k = np.arange(K)[None, None, :]
    ind = (k == NT * t + p // M)
    # gdr: +1 on min-path tiles, -0.5 on abs-path tiles
    wt = np.where(t >= NT - NA, -0.5, 1.0)
    gdr = (ind * wt).astype(ml_dtypes.bfloat16)
    gdr2 = ind.astype(ml_dtypes.bfloat16)

    identB = np.eye(64, dtype=ml_dtypes.bfloat16)
    identF = np.eye(128, dtype=np.float32)

    in_maps = []
    for c in range(NCORES):
        rows = slice(c * RPC, (c + 1) * RPC)
        cols = (c * RPC + np.arange(EXT)) % B
        xTr_c = np.ascontiguousarray(xT[:, cols]).astype(ml_dtypes.bfloat16)
        upper = c >= NCORES // 2
        biasD_c = np.full((64, 1), -100.0 if upper else 0.0, dtype=np.float32)
        in_maps.append({
            "xTr": xTr_c,
            "w2": w2,
            "xi": np.ascontiguousarray(x[rows]),
            "brep": brep,
            "gdr": gdr,
            "gdr2": gdr2,
            "identB": identB,
            "identF": identF,
            "biasD": biasD_c,
        })
    return in_maps


def kernel(x, W, b, _trace=False, _tmpdir=None):
    nc = _get_nc()
    in_maps = make_in_maps(x, W, b)
    res = bass_utils.run_bass_kernel_spmd(
        nc, in_maps, core_ids=list(range(NCORES)),
        trace=_trace, tmpdir=_tmpdir,
    )
    out = np.concatenate(
        [res.results[c]["out"] for c in range(NCORES)], axis=0)
    # host-side reduce of the column partials + self-term fix
    feats_add = np.zeros((B, K), dtype=np.float32)
    for c in range(NCORES):
        cadd = res.results[c]["outc"]                # [K, ACC]
        idx = (c * RPC + np.arange(ACC)) % B
        np.add.at(feats_add, idx, cadd.T)
    out[:, D:D + K] += feats_add + 1.0               # exact self term for every row
    if _trace:
        return out, res
    return out
